# revision 55
# baseline (speedup 1.0000x reference)
"""MoE (8 experts, top-2 routing) kernel for Trainium2 — 8 NeuronCores.

Sharding: expert-pair parallel with H-split.  Experts are paired
big-count-with-small-count; pair p is served by cores 2p (H-half 0) and
2p+1 (H-half 1).  Each core runs BOTH experts of its pair over the
pair's full token groups, but only its half of the hidden dimension
(H/2 = 2048), producing partial y that the host sums.  This balances
tokens across cores (~2111/core vs 2*1152 for naive expert-parallel)
while keeping weight traffic identical (each core holds half of two
experts = one expert's worth of weights).  bf16 operands double the
effective tensor-engine rate vs float32r on this part (f32r matmuls
run ~2.0 GHz-equivalent with a 16 ns/instr overhead; bf16 sustains
the full 2.4 GHz with ~3 ns/instr).

The small gate runs host-side; the host gathers tokens per expert into
the common SPMD capacities (Ca = max big-expert count, Cb = max small
count, exact, not 128-padded), and scatter-adds the 16 partial outputs
(2 cores x 2 experts per token) back to token order, applying the
combine weights and the b2 term there (exact for any b1/b2).

Device kernel (per core), all matmuls bf16 (fp32 PSUM accumulation):
  for e in (a, b):
    mm1 (token-tile-major, tiles paired per W1 chunk so each weight
        load covers two matmuls): hT[mi][:, tile] = relu(W1_mi^T x^T
        + b1) -> bf16
    mm2 (128-token PSUM tiles): py[t] = sum_mi hT[mi][:, t]^T @ W2_mi
        — the whole H/2 contraction accumulates in one PSUM residency;
        each hts weight load serves both 512-wide halves of D.
    flush: plain PSUM->SBUF copy on the scalar engine; stores alternate
        between the two HWDGE queues (the final store splits across
        both queues to shorten the tail).
Weights stream through SBUF exactly once; W1 of the running expert is
fully resident (16 KB/partition bf16).  DMA layouts are packed so
every load is a contiguous multi-KB burst per partition (W1 chunk-
major [P, MH, DK, P]; x tile-blocked [P, DK*tsz])."""

import numpy as np

P = 128
D = 1024
H = 4096
HH = H // 2       # per-core hidden half
E = 8
TOPK = 2
DK = D // P       # 8  contraction chunks for mm1
MH = HH // P      # 16 hidden chunks per core per expert


def _mm1_tiles(C):
    """Split C tokens into equal matmul moving-dim chunks (<=512 for one
    PSUM bank; chunks of ~340+ keep the bf16 matmul ahead of its 107 ns
    LDWEIGHTS so the PE is row-paced, not weight-load-paced)."""
    n = max(1, -(-C // 512))
    base, r = divmod(C, n)
    return [base + (1 if i < r else 0) for i in range(n)]


def _build_program(Ca, Cb):
    import concourse.mybir as mybir
    import concourse.tile as tile
    from concourse import bacc

    f32 = mybir.dt.float32
    bf16 = mybir.dt.bfloat16
    Relu = mybir.ActivationFunctionType.Relu
    Copy = mybir.ActivationFunctionType.Copy
    tts_a = _mm1_tiles(Ca)
    tts_b = _mm1_tiles(Cb)

    nc = bacc.Bacc(
        "TRN2",
        target_bir_lowering=False,
        debug=False,
        enable_asserts=False,
        num_devices=E,
    )
    # x tile-blocked: [P, sum_t DK*tsz_t] with per-tile [DK, tsz] blocks so
    # each tile loads as one contiguous multi-KB burst per partition
    xa_d = nc.dram_tensor("xa", [P, DK * Ca], bf16, kind="ExternalInput").ap()
    xb_d = nc.dram_tensor("xb", [P, DK * Cb], bf16, kind="ExternalInput").ap()
    # W1 chunk-major: [P, MH, DK, P] so each per-chunk DMA moves a
    # contiguous 2 KB per partition (full-rate DMA bursts)
    w1a_d = nc.dram_tensor("w1a", [P, MH, DK, P], bf16, kind="ExternalInput").ap()
    w1b_d = nc.dram_tensor("w1b", [P, MH, DK, P], bf16, kind="ExternalInput").ap()
    w2a_d = nc.dram_tensor("w2a", [HH, D], bf16, kind="ExternalInput").ap()
    w2b_d = nc.dram_tensor("w2b", [HH, D], bf16, kind="ExternalInput").ap()
    b1a_d = nc.dram_tensor("b1a", [P, MH], f32, kind="ExternalInput").ap()
    b1b_d = nc.dram_tensor("b1b", [P, MH], f32, kind="ExternalInput").ap()
    # the host applies the combine weights during scatter-add
    ya_d = nc.dram_tensor("ya", [Ca, D], f32, kind="ExternalOutput").ap()
    yb_d = nc.dram_tensor("yb", [Cb, D], f32, kind="ExternalOutput").ap()

    with tile.TileContext(nc) as tc:
        with (
            tc.tile_pool(name="const", bufs=1) as const,
            tc.tile_pool(name="w1p", bufs=2) as w1p,
            tc.tile_pool(name="w2p", bufs=24) as w2p,
            tc.tile_pool(name="htp", bufs=MH) as htp,
            tc.tile_pool(name="ysp", bufs=3) as ysp,
            tc.tile_pool(name="php", bufs=4, space="PSUM") as php,
            tc.tile_pool(name="pyp", bufs=2, space="PSUM") as pyp,
        ):
            # ---- SBUF destination tiles -------------------------------
            # W1 of each expert lives fully in SBUF while its mm1 runs
            # (16 KB/partition bf16); per-m-chunk DMAs so the tensor
            # engine can chase the stream from the first chunk.
            w1t = {}
            b1a = const.tile([P, MH], f32)
            b1b = const.tile([P, MH], f32)

            def load_x_tile(x_d, which, ti, off, tsz):
                t = const.tile([P, DK, tsz], bf16, name=f"x_{which}_{ti}")
                nc.sync.dma_start(t[:], x_d[:, DK * off:DK * (off + tsz)])
                return t

            def load_w2(w2_d, mi, name):
                t = w2p.tile([P, D], bf16, tag="w2", name=name)
                nc.sync.dma_start(t[:], w2_d[mi * P:(mi + 1) * P, :])
                return t

            # ---- DMA emission order (in-order SP queue, first-use order;
            # the ACT HWDGE queue moves bulk data too slowly for loads).
            # W1a goes per-chunk so the tensor engine chases the stream
            # through mm1 tile0.
            ta = w1p.tile([P, MH, DK, P], bf16, tag="w1", name="w1_a")
            w1t["a"] = ta
            xa_tiles = [load_x_tile(xa_d, "a", 0, 0, tts_a[0])]
            for mi in range(4):
                nc.sync.dma_start(ta[:, mi], w1a_d[:, mi])
            nc.sync.dma_start(b1a[:], b1a_d[:])
            off = tts_a[0]
            if len(tts_a) > 1:
                xa_tiles.append(load_x_tile(xa_d, "a", 1, off, tts_a[1]))
                off += tts_a[1]
            for mi in range(4, MH):
                nc.sync.dma_start(ta[:, mi], w1a_d[:, mi])
            for ti, tsz in enumerate(tts_a[2:], start=2):
                xa_tiles.append(load_x_tile(xa_d, "a", ti, off, tsz))
                off += tsz
            w2a = [load_w2(w2a_d, mi, f"w2a_{mi}") for mi in range(MH)]
            nc.sync.dma_start(b1b[:], b1b_d[:])
            xb_tiles = []
            off = 0
            for ti, tsz in enumerate(tts_b):
                xb_tiles.append(load_x_tile(xb_d, "b", ti, off, tsz))
                off += tsz
            tb = w1p.tile([P, MH, DK, P], bf16, tag="w1", name="w1_b")
            for s in range(0, MH, 4):
                nc.sync.dma_start(tb[:, s:s + 4], w1b_d[:, s:s + 4])
            w1t["b"] = tb
            w2b = [load_w2(w2b_d, mi, f"w2b_{mi}") for mi in range(MH)]

            # ---- PE warm-up on a zeroed tile: ramps the DVFS while the
            # first operands stream in.
            warm = const.tile([P, P], bf16)
            nc.any.memset(warm[:], 0.0)
            pw = php.tile([P, 512], f32, tag="ph", name="pw")
            # ~2us of spin: ramps the DVFS clock while the first operands
            # stream in on the two DMA queues
            for _ in range(16):
                nc.tensor.matmul(
                    pw[:, :P], warm[:], warm[:], start=True, stop=True
                )

            # ---- per-expert compute -----------------------------------
            def expert(which, C, tts, b1t, w2s, y_d):
                w1s = w1t[which]
                x_tiles = xa_tiles if which == "a" else xb_tiles
                hts = [
                    htp.tile([P, Ca], bf16, tag="ht", name=f"ht_{which}_{mi}")
                    for mi in range(MH)
                ]
                # mm1: token tiles processed in pairs per weight chunk —
                # each W1 LDWEIGHTS serves two matmuls (into two PSUM
                # banks), keeping the weight load fully hidden.  Tile 0
                # runs alone so compute starts on minimal input data.
                offs = []
                off = 0
                for tsz in tts:
                    offs.append(off)
                    off += tsz
                start = len(tts) % 2          # odd count: tile 0 solo first
                groups = [(0,)] * start + [
                    tuple(range(i, i + 2)) for i in range(start, len(tts), 2)
                ]
                for grp in groups:
                    for mi in range(MH):
                        phs = [
                            php.tile([P, 512], f32, tag="ph",
                                     name=f"ph_{which}_{grp[0]}_{mi}_{g}")
                            for g in range(len(grp))
                        ]
                        for dk in range(DK):
                            for g, ti in enumerate(grp):
                                nc.tensor.matmul(
                                    phs[g][:, :tts[ti]],
                                    w1s[:, mi, dk, :],
                                    x_tiles[ti][:, dk, :],
                                    start=(dk == 0),
                                    stop=(dk == DK - 1),
                                )
                        for g, ti in enumerate(grp):
                            nc.scalar.activation(
                                hts[mi][:, offs[ti]:offs[ti] + tts[ti]],
                                phs[g][:, :tts[ti]], Relu,
                                bias=b1t[:, mi:mi + 1],
                            )
                # mm2: whole H/2 contraction in one PSUM residency per
                # 128-token tile; each hts weight load serves 2 matmuls
                # (h2 halves) so LDWEIGHTS stays hidden.  Plain copy
                # flush — combine weights are applied host-side.
                T = -(-C // P)
                for t in range(T):
                    np_ = min(P, C - t * P)
                    py = pyp.tile([P, D], f32, tag="py")
                    for mi in range(MH):
                        for h2 in range(2):
                            nc.tensor.matmul(
                                py[:np_, h2 * 512:(h2 + 1) * 512],
                                hts[mi][:, t * P:t * P + np_],
                                w2s[mi][:, h2 * 512:(h2 + 1) * 512],
                                start=(mi == 0),
                                stop=(mi == MH - 1),
                            )
                    ys = ysp.tile([P, D], f32, tag="ys")
                    nc.scalar.activation(ys[:np_], py[:np_], Copy)
                    if which == "b" and t == T - 1:
                        # final store split across both queues
                        nc.sync.dma_start(
                            y_d[t * P:t * P + np_, 0:512], ys[:np_, 0:512]
                        )
                        nc.scalar.dma_start(
                            y_d[t * P:t * P + np_, 512:1024], ys[:np_, 512:1024]
                        )
                    else:
                        # alternate the two HWDGE queues so consecutive
                        # tile stores overlap instead of serializing
                        eng = nc.sync if t % 2 == 0 else nc.scalar
                        eng.dma_start(y_d[t * P:t * P + np_, :], ys[:np_])

            expert("a", Ca, tts_a, b1a, w2a, ya_d)
            expert("b", Cb, tts_b, b1b, w2b, yb_d)
    nc.compile()
    return nc


def _route(x, Wg, bg):
    """Host gate: softmax over experts + stable top-2 (mirrors
    jax.lax.top_k tie-breaking: lowest index first)."""
    logits = x @ Wg + bg
    mx = logits.max(axis=1, keepdims=True)
    ex = np.exp(logits - mx)
    gate = ex / ex.sum(axis=1, keepdims=True)
    top2 = np.argsort(-gate, axis=1, kind="stable")[:, :TOPK]
    return gate, top2


def _pack_x(x, idx, C, bf16):
    """Gathered tokens -> [P, DK*C] bf16, tile-blocked: per mm1 tile a
    contiguous [DK, tsz] block per partition (single-burst DMA loads)."""
    xe = np.zeros((C, D), np.float32)
    xe[: len(idx)] = x[idx]
    blocks = []
    off = 0
    for tsz in _mm1_tiles(C):
        blk = xe[off:off + tsz].T.reshape(DK, P, tsz).transpose(1, 0, 2)
        blocks.append(blk.reshape(P, DK * tsz))
        off += tsz
    return np.ascontiguousarray(np.concatenate(blocks, axis=1)).astype(bf16)


def kernel(x, Wg, bg, W1, b1, W2, b2):
    import ml_dtypes
    from concourse.bass_utils import run_bass_kernel_spmd

    bf16 = ml_dtypes.bfloat16
    x = np.asarray(x, np.float32)
    Wg = np.asarray(Wg, np.float32)
    bg = np.asarray(bg, np.float32)
    W1 = np.asarray(W1, np.float32)
    b1 = np.asarray(b1, np.float32)
    W2 = np.asarray(W2, np.float32)
    b2 = np.asarray(b2, np.float32)
    Ttok = x.shape[0]

    gate, top2 = _route(x, Wg, bg)
    expert_idx = [
        np.nonzero((top2 == e).any(axis=1))[0] for e in range(E)
    ]
    cnts = np.array([len(s) for s in expert_idx])
    order = np.argsort(-cnts, kind="stable")
    bigs = order[:4]
    smalls = order[4:][::-1]          # pair i-th largest with i-th smallest
    # exact SPMD capacities (mm1 cost scales with C; only the wc layout
    # and mm2 tile count are 128-granular)
    Ca = max(P * 2, int(cnts[bigs].max()))
    Cb = max(P * 2, int(cnts[smalls].max()))

    nc = _build_program(Ca, Cb)

    in_maps = []
    for p in range(4):
        ea, eb = int(bigs[p]), int(smalls[p])
        ia, ib = expert_idx[ea], expert_idx[eb]
        xa = _pack_x(x, ia, Ca, bf16)
        xb = _pack_x(x, ib, Cb, bf16)
        for half in range(2):
            hs = slice(half * HH, (half + 1) * HH)
            w1a = np.ascontiguousarray(
                W1[ea][:, hs].reshape(DK, P, MH, P).transpose(1, 2, 0, 3)
            ).astype(bf16)
            w1b = np.ascontiguousarray(
                W1[eb][:, hs].reshape(DK, P, MH, P).transpose(1, 2, 0, 3)
            ).astype(bf16)
            in_maps.append({
                "xa": xa, "xb": xb,
                "w1a": w1a, "w1b": w1b,
                "w2a": np.ascontiguousarray(W2[ea][hs, :]).astype(bf16),
                "w2b": np.ascontiguousarray(W2[eb][hs, :]).astype(bf16),
                "b1a": np.ascontiguousarray(b1[ea][hs].reshape(MH, P).T),
                "b1b": np.ascontiguousarray(b1[eb][hs].reshape(MH, P).T),
            })

    results = run_bass_kernel_spmd(nc, in_maps, core_ids=list(range(E))).results

    out = np.zeros((Ttok, D), np.float32)
    for p in range(4):
        ea, eb = int(bigs[p]), int(smalls[p])
        ia, ib = expert_idx[ea], expert_idx[eb]
        wa = gate[ia, ea][:, None]
        wb = gate[ib, eb][:, None]
        for half in range(2):
            r = results[2 * p + half]
            out[ia] += wa * r["ya"][: len(ia)]
            out[ib] += wb * r["yb"][: len(ib)]
    # b2 contribution, folded on the host (exact for any b2)
    mask = np.zeros((Ttok, E), np.float32)
    np.put_along_axis(mask, top2, 1.0, axis=1)
    out += (gate * mask) @ b2
    return out
